# revision 2
# baseline (speedup 1.0000x reference)
"""Trainium2 Bass kernel v2 for nn_Attention_44143673868291.

Data-parallel over batch (core b = batch b). Differences vs v1:
  - exp tiles are [128,1536] (3 psum banks, x2 ping-pong = 6 banks): 172
    ACT instructions instead of 256 (ACT exp is the wall).
  - O/r psum pair (2 banks) drains UNNORMALIZED to SBUF (bank released
    after one copy); recip + divide deferred; r-row alignment via a tiny
    SBUF->SBUF DMA partition shift (no PE shift-matmul, no psum hold).
  - out-projection in bf16 (wp/bp/outT were fp32 = 4 cy/row on PE).
  - rstd = exp(-0.5*ln(var+eps)): Ln+Exp share one ACT table set (no
    Sqrt switch); exp tables load during the LN phase.
  - triangular S-chunk schedule: attention g0 starts after the first
    q/k projection chunk; qk(g1) runs in psum-slot windows between O
    phases. Projections share the 2 "slot" psum banks with O.
  - gamma/beta/sqrt(SCALE) folded into weights host-side.
"""

import os
import sys

_REPO = "/opt/trn_rl_repo"
if _REPO not in sys.path:
    sys.path.insert(0, _REPO)

import numpy as np
import ml_dtypes

import concourse.bass as bass
import concourse.mybir as mybir
import concourse.bacc as bacc
import concourse.tile as tile
from concourse import bass_utils

F32 = mybir.dt.float32
F32R = mybir.dt.float32r
BF16 = mybir.dt.bfloat16
BF16_NP = ml_dtypes.bfloat16
Alu = mybir.AluOpType
Act = mybir.ActivationFunctionType

B, N, DIM, POS, H = 8, 2048, 256, 128, 8
QK = DIM + POS  # 384
HD = DIM // H   # 32
SCALE = HD ** -0.5
EPS = 1e-5
IBS = 512
TT = N // 128   # 16 token tiles
JT = N // 128   # 16 j tiles
IB = N // IBS   # 4 i-blocks per head-group
CHUNK = 512     # columns per S-chunk (one head x one i-block x one j-tile)
STW = 3         # S-chunks per exp tile (1536 cols)

# bank pair layout: bankA holds heads (4g+0, 4g+2), bankB holds (4g+3, 4g+1)
#   h0->(A,0) h1->(B,64) h2->(A,64) h3->(B,0)
H_BANK = (0, 1, 0, 1)   # which bank (A=0/B=1) per h%4
H_POS = (0, 64, 64, 0)  # column/tile position per h%4
PAIR_HEADS = [(0, 2), (3, 1), (4, 6), (7, 5)]


def build_nc(n=N, repeat=1, ptbufs=26):
    nc = bacc.Bacc("TRN2", target_bir_lowering=False, debug=False)

    d = lambda name, shape, dt: nc.dram_tensor(name, shape, dt, kind="ExternalInput").ap()
    x_d = d("x", [n, DIM], F32)
    posT_d = d("posT", [POS, n], BF16)
    wq_d = d("wq", [QK, DIM], BF16)
    wk_d = d("wk", [QK, DIM], BF16)
    wv_d = d("wv", [DIM, DIM], BF16)
    wp_d = [d(f"wp{p}", [128, DIM], F32R) for p in range(4)]
    bq_d = d("bq", [DIM, 1], F32)
    bk_d = d("bk", [DIM, 1], F32)
    bv_d = d("bv", [1, DIM], BF16)
    bp_d = d("bp", [1, DIM], F32R)
    onerb_d = d("ones_row_bf", [1, 128], BF16)
    onerf_d = d("ones_row_f", [1, 128], F32R)
    i128_d = d("i128", [128, 128], F32)
    epsc_d = d("epsc", [128, 1], F32)
    zeroc_d = d("zeroc", [128, 1], F32)
    out_d = nc.dram_tensor("out", [n, DIM], F32, kind="ExternalOutput").ap()

    from contextlib import ExitStack

    with tile.TileContext(nc) as tc, ExitStack() as ctx:
        cp = ctx.enter_context(tc.tile_pool(name="const", bufs=1))

        def ctile(shape, dt, src, tag):
            t = cp.tile(shape, dt, tag=tag, name=tag)
            nc.sync.dma_start(t[:], src)
            return t

        # DMA order = need-time: x[0:4] (LN) -> early consts (transposes,
        # qk projections) -> rest of x -> late consts.
        xsb = [cp.tile([128, DIM], F32, tag=f"x{t}", name=f"x{t}") for t in range(TT)]
        for t in range(4):
            nc.sync.dma_start(xsb[t][:], x_d[128 * t:128 * (t + 1), :])
        epsc = ctile([128, 1], F32, epsc_d[:, :], "epsc")
        zeroc = ctile([128, 1], F32, zeroc_d[:, :], "zeroc")
        i128 = ctile([128, 128], F32, i128_d[:, :], "i128")
        wq = [[ctile([128, 128], BF16, wq_d[128 * k:128 * (k + 1), 128 * g:128 * (g + 1)],
                     f"wq{k}{g}") for g in range(2)] for k in range(3)]
        wk = [[ctile([128, 128], BF16, wk_d[128 * k:128 * (k + 1), 128 * g:128 * (g + 1)],
                     f"wk{k}{g}") for g in range(2)] for k in range(3)]
        bq = [ctile([128, 1], F32, bq_d[128 * g:128 * (g + 1), :], f"bq{g}") for g in range(2)]
        bk = [ctile([128, 1], F32, bk_d[128 * g:128 * (g + 1), :], f"bk{g}") for g in range(2)]
        posT = ctile([POS, n], BF16, posT_d[:, :], "posT")
        for t in range(4, TT):
            nc.sync.dma_start(xsb[t][:], x_d[128 * t:128 * (t + 1), :])
        wv = [ctile([128, DIM], BF16, wv_d[128 * k:128 * (k + 1), :], f"wv{k}") for k in range(2)]
        bv = ctile([1, DIM], BF16, bv_d[:, :], "bv")
        onerb = ctile([1, 128], BF16, onerb_d[:, :], "onerb")
        onerf = ctile([1, 128], F32R, onerf_d[:, :], "onerf")
        wp = [ctile([128, DIM], F32R, wp_d[p][:, :], f"wp{p}") for p in range(4)]
        bp = ctile([1, DIM], F32R, bp_d[:, :], "bp")

        # persistent activations
        xnT = [cp.tile([128, n], BF16, tag=f"xnT{g}", name=f"xnT{g}") for g in range(2)]
        qT = [cp.tile([128, n], BF16, tag=f"qT{g}", name=f"qT{g}") for g in range(2)]
        kT = [cp.tile([128, n], BF16, tag=f"kT{g}", name=f"kT{g}") for g in range(2)]
        # all-t augmented V: col = 512*t + 64*h + c, c<32 = v, c>=32 = ones
        vsb = cp.tile([128, TT * 512], BF16, tag="vsb", name="vsb")
        nc.vector.memset(vsb[:], 1.0)
        outT = [cp.tile([128, n], F32R, tag=f"outT{p}", name=f"outT{p}") for p in range(4)]
        stats = cp.tile([128, 2 * TT], F32, tag="stats", name="stats")
        lnv = cp.tile([128, TT], F32, tag="lnv", name="lnv")
        rstd = cp.tile([128, TT], F32, tag="rstd", name="rstd")
        vz = cp.tile([128, TT], F32, tag="vz", name="vz")
        nt1 = cp.tile([128, TT], F32, tag="nt1", name="nt1")
        nt2 = cp.tile([128, TT], F32, tag="nt2", name="nt2")

        bn6p = ctx.enter_context(tc.tile_pool(name="bn6", bufs=3))
        xcp = ctx.enter_context(tc.tile_pool(name="xc", bufs=6))
        ptp = ctx.enter_context(tc.tile_pool(name="pt", bufs=ptbufs))
        ocp = ctx.enter_context(tc.tile_pool(name="oc", bufs=2))
        rshp = ctx.enter_context(tc.tile_pool(name="rsh", bufs=2))
        fp = ctx.enter_context(tc.tile_pool(name="fout", bufs=3))
        # psum: slots 2 banks + st 6 banks = 8
        op = ctx.enter_context(tc.tile_pool(name="slots", bufs=1, space="PSUM"))
        stp = ctx.enter_context(tc.tile_pool(name="st", bufs=2, space="PSUM"))

        slot = lambda i: op.tile([128, 512], F32, tag=f"s{'AB'[i]}", name=f"s{'AB'[i]}")

        for _rep in range(repeat):
            # ---------------- LN stats ----------------
            for t in range(TT):
                if _rep > 0:
                    nc.sync.dma_start(xsb[t][:], x_d[128 * t:128 * (t + 1), :])
                b6 = bn6p.tile([128, 6], F32, tag="b6", name="b6")
                nc.vector.bn_stats(b6[:], xsb[t][:])
                nc.vector.bn_aggr(stats[:, 2 * t:2 * t + 2], b6[:])
            # rstd = exp(-0.5*ln(var+eps)); Ln+Exp share one ACT table set,
            # and this loads the exp tables before the big softmax exps.
            QB = max(TT // 4, 1)
            for qb in range(0, TT, QB):
                qn = min(QB, TT - qb)
                var_v = stats[:, 2 * qb:2 * (qb + qn)].rearrange(
                    "p (t c) -> p t c", c=2)[:, :, 1:2]
                lnv_v = lnv[:, qb:qb + qn].rearrange("p (t c) -> p t c", c=1)
                rstd_v = rstd[:, qb:qb + qn].rearrange("p (t c) -> p t c", c=1)
                nc.scalar.activation(lnv_v, var_v, Act.Ln, bias=epsc[:])
                nc.scalar.activation(rstd_v, lnv_v, Act.Exp, bias=zeroc[:], scale=-0.5)
                vz_v = vz[:, qb:qb + qn].rearrange("p (t c) -> p t c", c=1)
                nc.vector.tensor_scalar(vz_v, var_v, epsc[:], None, op0=Alu.add)
            # one Newton step squares away the ACT-table error:
            # rstd <- rstd * (1.5 - 0.5 * (var+eps) * rstd^2)
            nc.vector.tensor_mul(nt1[:], rstd[:], rstd[:])
            nc.vector.tensor_mul(nt1[:], nt1[:], vz[:])
            nc.vector.tensor_mul(nt1[:], nt1[:], rstd[:])
            nc.vector.tensor_scalar(nt2[:], rstd[:], 1.5, None, op0=Alu.mult)
            nc.vector.tensor_scalar(nt1[:], nt1[:], 0.5, None, op0=Alu.mult)
            nc.vector.tensor_sub(rstd[:], nt2[:], nt1[:])

            # ---------------- chunk/exp stream machinery ----------------
            st_state = {"tile": None, "fill": 0, "cap": 1}  # first tile: 1 chunk
            # O-phase state: one (g, ib) pair owns the two os slot banks at a
            # time; its O-chunks are emitted incrementally as exp tiles flush.
            ost = {
                "order": [(g, ib) for g in range(2) for ib in range(IB)],
                "head": 0,            # index into order: current O owner
                "ready": {},          # (g, ib) -> list of (jt, h, pt, off)
                "done": {},           # (g, ib) -> count of emitted O-chunks
                "banks": None,        # (osA, osB) tiles of current owner
                "bcount": [0, 0],     # chunks emitted per bank for owner
                "windows": {},        # close-of-(g,ib) -> list of (g2, c) qk
            }

            def emit_O_chunk(g, ib, jt, h, pt, off):
                if ost["banks"] is None:
                    ost["banks"] = (slot(0), slot(1))
                    nc.vector.memset(ost["banks"][0][:], 0.0)
                    nc.vector.memset(ost["banks"][1][:], 0.0)
                bi, pos = H_BANK[h], H_POS[h]
                ost["bcount"][bi] += 1
                nc.tensor.matmul(
                    ost["banks"][bi][pos:pos + 64, :],
                    lhsT=vsb[:, 512 * jt + 64 * (4 * g + h):
                             512 * jt + 64 * (4 * g + h) + 64],
                    rhs=pt[:, CHUNK * off:CHUNK * (off + 1)],
                    start=False,
                    stop=ost["bcount"][bi] == 2 * JT,
                    tile_position=(0, pos), skip_group_check=True)

            def emit_norm(g, ib):
                ibs = slice(IBS * ib, IBS * (ib + 1))
                for bi, bank in enumerate(ost["banks"]):
                    oc = ocp.tile([128, IBS], F32, tag="oc", name="oc")
                    nc.vector.tensor_copy(oc[:], bank[:])  # releases the bank
                    rsh = rshp.tile([128, IBS], F32, tag="rsh", name="rsh")
                    nc.vector.memset(rsh[:], 1.0)
                    nc.sync.dma_start(rsh[0:32, :], oc[32:64, :])
                    nc.sync.dma_start(rsh[64:96, :], oc[96:128, :])
                    rr = rshp.tile([128, IBS], F32, tag="rr", name="rr")
                    nc.vector.memset(rr[:], 1.0)
                    nc.vector.reciprocal_approx_fast(rr[:], rsh[:])
                    nc.vector.tensor_mul(outT[2 * g + bi][:, ibs], oc[:], rr[:])
                ost["banks"] = None
                ost["bcount"] = [0, 0]

            def emit_phase5(ib):
                # out-projection + residual for the 4 token tiles of this ib;
                # legal once both head-groups' norms for ib are done.
                for t in range(4 * ib, 4 * ib + 4):
                    ts_ = slice(128 * t, 128 * (t + 1))
                    f_ps = slot(t % 2)
                    for p in range(4):
                        nc.tensor.matmul(f_ps[:, :DIM], lhsT=outT[p][:, ts_],
                                         rhs=wp[p][:], start=(p == 0), stop=False)
                    nc.tensor.matmul(f_ps[:, :DIM], lhsT=onerf[:], rhs=bp[:],
                                     start=False, stop=True)
                    f_sb = fp.tile([128, DIM], F32, tag="f", name="f")
                    nc.vector.tensor_add(f_sb[:], f_ps[:, :DIM], xsb[t][:])
                    nc.sync.dma_start(out_d[ts_, :], f_sb[:])

            def drain_O():
                # emit O-chunks for the current owner; on completion, norm +
                # fire any queued projection window, then advance the owner.
                while ost["head"] < len(ost["order"]):
                    g, ib = ost["order"][ost["head"]]
                    rl = ost["ready"].get((g, ib), [])
                    done = ost["done"].get((g, ib), 0)
                    for jt, h, pt, off in rl[done:]:
                        emit_O_chunk(g, ib, jt, h, pt, off)
                    ost["done"][(g, ib)] = len(rl)
                    if len(rl) < 4 * JT:
                        return
                    emit_norm(g, ib)
                    for g2, c in ost["windows"].pop((g, ib), []):
                        emit_tpose_qk_v(g2, c)
                    if g == 1:
                        emit_phase5(ib)
                    ost["head"] += 1

            def flush_exp():
                stt, fill = st_state["tile"], st_state["fill"]
                if stt is None or fill == 0:
                    return
                pt = ptp.tile([128, STW * CHUNK], BF16, tag="pt", name="pt")
                nc.scalar.activation(pt[:, :fill * CHUNK], stt[0][:, :fill * CHUNK],
                                     Act.Exp, bias=zeroc[:])
                for (gib, jt, h, off) in stt[1]:
                    ost["ready"].setdefault(gib, []).append((jt, h, pt, off))
                st_state["tile"] = None
                st_state["fill"] = 0
                st_state["cap"] = STW
                drain_O()

            def emit_S(g, ib, jt):
                for h in range(4):
                    if st_state["tile"] is None:
                        st_state["tile"] = (
                            stp.tile([128, STW * CHUNK], F32, tag="st", name="st"), [])
                    stt, fill = st_state["tile"], st_state["fill"]
                    cs = slice(CHUNK * fill, CHUNK * (fill + 1))
                    nc.tensor.matmul(
                        stt[0][:, cs],
                        lhsT=kT[g][32 * h:32 * (h + 1), 128 * jt:128 * (jt + 1)],
                        rhs=qT[g][32 * h:32 * (h + 1), IBS * ib:IBS * (ib + 1)],
                        start=True, stop=True, tile_position=(32 * h, 0))
                    stt[1].append(((g, ib), jt, h, fill))
                    st_state["fill"] = fill + 1
                    if st_state["fill"] == st_state["cap"]:
                        flush_exp()

            def emit_v_pair(ptile, base, t0):
                # v for tokens t0, t0+1 into ptile[:, base:base+512]
                for ti in range(2):
                    t = t0 + ti
                    vs = slice(base + 256 * ti, base + 256 * (ti + 1))
                    nc.tensor.matmul(ptile[:, vs], lhsT=xnT[0][:, 128 * t:128 * (t + 1)],
                                     rhs=wv[0][:], start=True, stop=False)
                    nc.tensor.matmul(ptile[:, vs], lhsT=xnT[1][:, 128 * t:128 * (t + 1)],
                                     rhs=wv[1][:], start=False, stop=False)
                    nc.tensor.matmul(ptile[:, vs], lhsT=onerb[:], rhs=bv[:],
                                     start=False, stop=True)
                vdst = vsb[:, 512 * t0:512 * (t0 + 2)].rearrange(
                    "p (t hh cc) -> p t hh cc", t=2, cc=64)[:, :, :, 0:32]
                vsrc = ptile[:, base:base + 512].rearrange(
                    "p (t hh cc) -> p t hh cc", t=2, cc=32)
                nc.vector.tensor_copy(vdst, vsrc)

            def emit_tpose_qk_v(g2, c):
                # g2: head-group whose qk to project; c: column chunk 0..3.
                # Projections borrow st-pool tiles (512-col bank slices) so the
                # os banks stay exclusive to O accumulation. For g2==0, two
                # tiles per c (even count keeps the S/exp ping-pong parity);
                # window calls (g2==1) come in pairs for the same reason.
                if g2 == 0:
                    xcs = []
                    for t in range(4 * c, 4 * c + 4):
                        xc = xcp.tile([128, DIM], F32, tag="xc", name="xc")
                        nc.vector.tensor_scalar(
                            xc[:], xsb[t][:], stats[:, 2 * t:2 * t + 1],
                            rstd[:, t:t + 1], op0=Alu.subtract, op1=Alu.mult)
                        xcs.append(xc)
                    pt1 = stp.tile([128, STW * CHUNK], F32, tag="st", name="st")
                    for gg in range(2):
                        for t4 in range(4):
                            nc.tensor.transpose(
                                pt1[:, 512 * gg + 128 * t4:512 * gg + 128 * (t4 + 1)],
                                xcs[t4][:, 128 * gg:128 * (gg + 1)],
                                i128[:])
                        nc.vector.tensor_copy(
                            xnT[gg][:, 512 * c:512 * (c + 1)],
                            pt1[:, 512 * gg:512 * (gg + 1)])
                cs = slice(512 * c, 512 * (c + 1))
                pt2 = stp.tile([128, STW * CHUNK], F32, tag="st", name="st")
                for which, (w, bias, dstT) in enumerate(
                        [(wq, bq, qT), (wk, bk, kT)]):
                    pj = pt2[:, 512 * which:512 * (which + 1)]
                    for ki in range(3):
                        rhs = xnT[ki][:, cs] if ki < 2 else posT[:, cs]
                        nc.tensor.matmul(pj, lhsT=w[ki][g2][:], rhs=rhs,
                                         start=(ki == 0), stop=(ki == 2))
                    nc.vector.tensor_scalar(dstT[g2][:, cs], pj, bias[g2][:],
                                            None, op0=Alu.add)
                if g2 == 0:
                    emit_v_pair(pt2, 1024, 4 * c)
                    emit_v_pair(pt1, 1024, 4 * c + 2)

            # ---------------- emission schedule ----------------
            # diagonal growth: round c emits S-chunks with ib + jt//4 == c,
            # so (0,0) completes early and O-draining starts immediately.
            ost["windows"][(0, 0)] = [(1, 0), (1, 1)]
            ost["windows"][(0, 1)] = [(1, 2), (1, 3)]
            for c in range(4):
                emit_tpose_qk_v(0, c)
                for ib in range(min(c + 1, IB)):
                    jq = c - ib
                    for jt in range(4 * jq, 4 * jq + 4):
                        emit_S(0, ib, jt)
            # g0 remainder: ib + jt//4 >= 4, closing ibs in order
            for ib in range(1, IB):
                for jq in range(4 - ib, 4):
                    for jt in range(4 * jq, 4 * jq + 4):
                        emit_S(0, ib, jt)
            # g1 (windows fired inline at close(0,0) and close(0,1))
            for ib in range(2):
                for jt in range(8):
                    emit_S(1, ib, jt)
            for ib in range(2):
                for jt in range(8, JT):
                    emit_S(1, ib, jt)
            for ib in range(2, IB):
                for jt in range(JT):
                    emit_S(1, ib, jt)
            flush_exp()  # phase5 emitted inline by drain_O at each g1 norm

    nc.compile()
    return nc


def make_in_maps(inputs, n=N, nb=B):
    x = np.ascontiguousarray(np.asarray(inputs["x"], np.float32))
    pos = np.asarray(inputs["pos_embed"], np.float32)
    f32 = lambda a: np.ascontiguousarray(np.asarray(a, np.float32))
    bf16 = lambda a: np.ascontiguousarray(np.asarray(a, np.float32).astype(BF16_NP))

    g = f32(inputs["ln_g"]).reshape(DIM)
    b = f32(inputs["ln_b"]).reshape(DIM)
    rs = SCALE ** 0.5  # sqrt of softmax scale, folded into q AND k
    Wq, Wk, Wv = f32(inputs["Wq"]), f32(inputs["Wk"]), f32(inputs["Wv"])
    wq_eff = Wq.copy()
    wq_eff[:DIM] *= g[:, None]
    wk_eff = Wk.copy()
    wk_eff[:DIM] *= g[:, None]
    bq_eff = (f32(inputs["bq"]) + Wq[:DIM].T @ b) * rs
    bk_eff = (f32(inputs["bk"]) + Wk[:DIM].T @ b) * rs
    wv_eff = Wv * g[:, None]
    bv_eff = f32(inputs["bv"]) + Wv.T @ b

    shared = {
        "wq": bf16(wq_eff * rs), "wk": bf16(wk_eff * rs), "wv": bf16(wv_eff),
        "bq": bq_eff.reshape(DIM, 1), "bk": bk_eff.reshape(DIM, 1),
        "bv": bf16(bv_eff).reshape(1, DIM), "bp": f32(inputs["bp"]).reshape(1, DIM),
        "ones_row_bf": np.ones((1, 128), BF16_NP),
        "ones_row_f": np.ones((1, 128), np.float32),
        "i128": np.eye(128, dtype=np.float32),
        "epsc": np.full((128, 1), EPS, np.float32),
        "zeroc": np.zeros((128, 1), np.float32),
    }
    wp_full = f32(inputs["Wp"])
    for p, (ha, hb) in enumerate(PAIR_HEADS):
        pad = np.zeros((128, DIM), np.float32)
        pad[0:32] = wp_full[32 * ha:32 * ha + 32]
        pad[64:96] = wp_full[32 * hb:32 * hb + 32]
        shared[f"wp{p}"] = pad
    in_maps = []
    for bb in range(nb):
        m = dict(shared)
        m["x"] = np.ascontiguousarray(x[bb, :n])
        m["posT"] = np.ascontiguousarray(pos[bb, :n].T.astype(BF16_NP))
        in_maps.append(m)
    return in_maps


_NC_CACHE = {}


def kernel(**inputs):
    if "nc" not in _NC_CACHE:
        _NC_CACHE["nc"] = build_nc()
    nc = _NC_CACHE["nc"]
    in_maps = make_in_maps(inputs)
    trace = bool(int(os.environ.get("KERNEL_TRACE", "0")))
    if not trace:
        os.environ["BASS_NEVER_TRACE"] = "1"
    res = bass_utils.run_bass_kernel_spmd(nc, in_maps, core_ids=list(range(B)), trace=trace)
    kernel.last_results = res
    kernel.last_exec_time_ns = res.exec_time_ns
    return np.stack([r["out"] for r in res.results]).astype(np.float32)


# revision 3
# speedup vs baseline: 1.0174x; 1.0174x over previous
"""Trainium2 Bass kernel v2 for nn_Attention_44143673868291.

Data-parallel over batch (core b = batch b). Differences vs v1:
  - exp tiles are [128,1536] (3 psum banks, x2 ping-pong = 6 banks): 172
    ACT instructions instead of 256 (ACT exp is the wall).
  - O/r psum pair (2 banks) drains UNNORMALIZED to SBUF (bank released
    after one copy); recip + divide deferred; r-row alignment via a tiny
    SBUF->SBUF DMA partition shift (no PE shift-matmul, no psum hold).
  - out-projection in bf16 (wp/bp/outT were fp32 = 4 cy/row on PE).
  - rstd = exp(-0.5*ln(var+eps)): Ln+Exp share one ACT table set (no
    Sqrt switch); exp tables load during the LN phase.
  - triangular S-chunk schedule: attention g0 starts after the first
    q/k projection chunk; qk(g1) runs in psum-slot windows between O
    phases. Projections share the 2 "slot" psum banks with O.
  - gamma/beta/sqrt(SCALE) folded into weights host-side.
"""

import os
import sys

_REPO = "/opt/trn_rl_repo"
if _REPO not in sys.path:
    sys.path.insert(0, _REPO)

import numpy as np
import ml_dtypes

import concourse.bass as bass
import concourse.mybir as mybir
import concourse.bacc as bacc
import concourse.tile as tile
from concourse import bass_utils

F32 = mybir.dt.float32
F32R = mybir.dt.float32r
BF16 = mybir.dt.bfloat16
BF16_NP = ml_dtypes.bfloat16
Alu = mybir.AluOpType
Act = mybir.ActivationFunctionType

B, N, DIM, POS, H = 8, 2048, 256, 128, 8
QK = DIM + POS  # 384
HD = DIM // H   # 32
SCALE = HD ** -0.5
EPS = 1e-5
IBS = 512
TT = N // 128   # 16 token tiles
JT = N // 128   # 16 j tiles
IB = N // IBS   # 4 i-blocks per head-group
CHUNK = 512     # columns per S-chunk (one head x one i-block x one j-tile)
STW = 3         # S-chunks per exp tile (1536 cols)

# bank pair layout: bankA holds heads (4g+0, 4g+2), bankB holds (4g+3, 4g+1)
#   h0->(A,0) h1->(B,64) h2->(A,64) h3->(B,0)
H_BANK = (0, 1, 0, 1)   # which bank (A=0/B=1) per h%4
H_POS = (0, 64, 64, 0)  # column/tile position per h%4
PAIR_HEADS = [(0, 2), (3, 1), (4, 6), (7, 5)]


def build_nc(n=N, repeat=1, ptbufs=26):
    nc = bacc.Bacc("TRN2", target_bir_lowering=False, debug=False)

    d = lambda name, shape, dt: nc.dram_tensor(name, shape, dt, kind="ExternalInput").ap()
    x_d = d("x", [n, DIM], F32)
    posT_d = d("posT", [POS, n], BF16)
    wq_d = d("wq", [QK, DIM], BF16)
    wk_d = d("wk", [QK, DIM], BF16)
    wv_d = d("wv", [DIM, DIM], BF16)
    wp_d = [d(f"wp{p}", [128, DIM], F32R) for p in range(4)]
    bq_d = d("bq", [DIM, 1], F32)
    bk_d = d("bk", [DIM, 1], F32)
    bv_d = d("bv", [1, DIM], BF16)
    bp_d = d("bp", [1, DIM], F32R)
    onerb_d = d("ones_row_bf", [1, 128], BF16)
    onerf_d = d("ones_row_f", [1, 128], F32R)
    i128_d = d("i128", [128, 128], F32)
    epsc_d = d("epsc", [128, 1], F32)
    zeroc_d = d("zeroc", [128, 1], F32)
    out_d = nc.dram_tensor("out", [n, DIM], F32, kind="ExternalOutput").ap()

    from contextlib import ExitStack

    with tile.TileContext(nc) as tc, ExitStack() as ctx:
        cp = ctx.enter_context(tc.tile_pool(name="const", bufs=1))

        def ctile(shape, dt, src, tag):
            t = cp.tile(shape, dt, tag=tag, name=tag)
            nc.sync.dma_start(t[:], src)
            return t

        # DMA order = need-time: x[0:4] (LN) -> early consts (transposes,
        # qk projections) -> rest of x -> late consts.
        xsb = [cp.tile([128, DIM], F32, tag=f"x{t}", name=f"x{t}") for t in range(TT)]
        for t in range(4):
            nc.sync.dma_start(xsb[t][:], x_d[128 * t:128 * (t + 1), :])
        epsc = ctile([128, 1], F32, epsc_d[:, :], "epsc")
        zeroc = ctile([128, 1], F32, zeroc_d[:, :], "zeroc")
        i128 = ctile([128, 128], F32, i128_d[:, :], "i128")
        wq = [[ctile([128, 128], BF16, wq_d[128 * k:128 * (k + 1), 128 * g:128 * (g + 1)],
                     f"wq{k}{g}") for g in range(2)] for k in range(3)]
        wk = [[ctile([128, 128], BF16, wk_d[128 * k:128 * (k + 1), 128 * g:128 * (g + 1)],
                     f"wk{k}{g}") for g in range(2)] for k in range(3)]
        bq = [ctile([128, 1], F32, bq_d[128 * g:128 * (g + 1), :], f"bq{g}") for g in range(2)]
        bk = [ctile([128, 1], F32, bk_d[128 * g:128 * (g + 1), :], f"bk{g}") for g in range(2)]
        posT = ctile([POS, n], BF16, posT_d[:, :], "posT")
        for t in range(4, TT):
            nc.sync.dma_start(xsb[t][:], x_d[128 * t:128 * (t + 1), :])
        wv = [ctile([128, DIM], BF16, wv_d[128 * k:128 * (k + 1), :], f"wv{k}") for k in range(2)]
        bv = ctile([1, DIM], BF16, bv_d[:, :], "bv")
        onerb = ctile([1, 128], BF16, onerb_d[:, :], "onerb")
        onerf = ctile([1, 128], F32R, onerf_d[:, :], "onerf")
        wp = [ctile([128, DIM], F32R, wp_d[p][:, :], f"wp{p}") for p in range(4)]
        bp = ctile([1, DIM], F32R, bp_d[:, :], "bp")

        # persistent activations
        xnT = [cp.tile([128, n], BF16, tag=f"xnT{g}", name=f"xnT{g}") for g in range(2)]
        qT = [cp.tile([128, n], BF16, tag=f"qT{g}", name=f"qT{g}") for g in range(2)]
        kT = [cp.tile([128, n], BF16, tag=f"kT{g}", name=f"kT{g}") for g in range(2)]
        # all-t augmented V: col = 512*t + 64*h + c, c<32 = v, c>=32 = ones
        vsb = cp.tile([128, TT * 512], BF16, tag="vsb", name="vsb")
        nc.vector.memset(vsb[:], 1.0)
        outT = [cp.tile([128, n], F32R, tag=f"outT{p}", name=f"outT{p}") for p in range(4)]
        stats = cp.tile([128, 2 * TT], F32, tag="stats", name="stats")
        lnv = cp.tile([128, TT], F32, tag="lnv", name="lnv")
        rstd = cp.tile([128, TT], F32, tag="rstd", name="rstd")
        vz = cp.tile([128, TT], F32, tag="vz", name="vz")
        nt1 = cp.tile([128, TT], F32, tag="nt1", name="nt1")
        nt2 = cp.tile([128, TT], F32, tag="nt2", name="nt2")

        bn6p = ctx.enter_context(tc.tile_pool(name="bn6", bufs=3))
        xcp = ctx.enter_context(tc.tile_pool(name="xc", bufs=6))
        ptp = ctx.enter_context(tc.tile_pool(name="pt", bufs=ptbufs))
        ocp = ctx.enter_context(tc.tile_pool(name="oc", bufs=2))
        rshp = ctx.enter_context(tc.tile_pool(name="rsh", bufs=2))
        fp = ctx.enter_context(tc.tile_pool(name="fout", bufs=3))
        # psum: slots 2 banks + st 6 banks = 8
        op = ctx.enter_context(tc.tile_pool(name="slots", bufs=1, space="PSUM"))
        stp = ctx.enter_context(tc.tile_pool(name="st", bufs=2, space="PSUM"))

        slot = lambda i: op.tile([128, 512], F32, tag=f"s{'AB'[i]}", name=f"s{'AB'[i]}")

        for _rep in range(repeat):
            # ---------------- LN stats ----------------
            for t in range(TT):
                if _rep > 0:
                    nc.sync.dma_start(xsb[t][:], x_d[128 * t:128 * (t + 1), :])
                b6 = bn6p.tile([128, 6], F32, tag="b6", name="b6")
                nc.vector.bn_stats(b6[:], xsb[t][:])
                nc.vector.bn_aggr(stats[:, 2 * t:2 * t + 2], b6[:])
            # rstd = exp(-0.5*ln(var+eps)); Ln+Exp share one ACT table set,
            # and this loads the exp tables before the big softmax exps.
            QB = max(TT // 4, 1)
            for qb in range(0, TT, QB):
                qn = min(QB, TT - qb)
                var_v = stats[:, 2 * qb:2 * (qb + qn)].rearrange(
                    "p (t c) -> p t c", c=2)[:, :, 1:2]
                lnv_v = lnv[:, qb:qb + qn].rearrange("p (t c) -> p t c", c=1)
                rstd_v = rstd[:, qb:qb + qn].rearrange("p (t c) -> p t c", c=1)
                nc.scalar.activation(lnv_v, var_v, Act.Ln, bias=epsc[:])
                nc.scalar.activation(rstd_v, lnv_v, Act.Exp, bias=zeroc[:], scale=-0.5)
                vz_v = vz[:, qb:qb + qn].rearrange("p (t c) -> p t c", c=1)
                nc.vector.tensor_scalar(vz_v, var_v, epsc[:], None, op0=Alu.add)
                # per-quarter Newton step squares away the ACT-table error
                # (per-quarter so xc(t0..3) doesn't wait on the full LN):
                # rstd <- rstd * (1.5 - 0.5 * (var+eps) * rstd^2)
                q_ = slice(qb, qb + qn)
                nc.vector.tensor_mul(nt1[:, q_], rstd[:, q_], rstd[:, q_])
                nc.vector.tensor_mul(nt1[:, q_], nt1[:, q_], vz[:, q_])
                nc.vector.tensor_mul(nt1[:, q_], nt1[:, q_], rstd[:, q_])
                nc.vector.tensor_scalar(nt2[:, q_], rstd[:, q_], 1.5, None, op0=Alu.mult)
                nc.vector.tensor_scalar(nt1[:, q_], nt1[:, q_], 0.5, None, op0=Alu.mult)
                nc.vector.tensor_sub(rstd[:, q_], nt2[:, q_], nt1[:, q_])

            # ---------------- chunk/exp stream machinery ----------------
            st_state = {"tile": None, "fill": 0, "cap": 1}  # first tile: 1 chunk
            # O-phase state: one (g, ib) pair owns the two os slot banks at a
            # time; its O-chunks are emitted incrementally as exp tiles flush.
            ost = {
                "order": [(g, ib) for g in range(2) for ib in range(IB)],
                "head": 0,            # index into order: current O owner
                "ready": {},          # (g, ib) -> list of (jt, h, pt, off)
                "done": {},           # (g, ib) -> count of emitted O-chunks
                "banks": None,        # (osA, osB) tiles of current owner
                "bcount": [0, 0],     # chunks emitted per bank for owner
                "windows": {},        # close-of-(g,ib) -> list of (g2, c) qk
            }

            def emit_O_chunk(g, ib, jt, h, pt, off):
                if ost["banks"] is None:
                    ost["banks"] = (slot(0), slot(1))
                    nc.vector.memset(ost["banks"][0][:], 0.0)
                    nc.vector.memset(ost["banks"][1][:], 0.0)
                bi, pos = H_BANK[h], H_POS[h]
                ost["bcount"][bi] += 1
                nc.tensor.matmul(
                    ost["banks"][bi][pos:pos + 64, :],
                    lhsT=vsb[:, 512 * jt + 64 * (4 * g + h):
                             512 * jt + 64 * (4 * g + h) + 64],
                    rhs=pt[:, CHUNK * off:CHUNK * (off + 1)],
                    start=False,
                    stop=ost["bcount"][bi] == 2 * JT,
                    tile_position=(0, pos), skip_group_check=True)

            def emit_norm(g, ib):
                ibs = slice(IBS * ib, IBS * (ib + 1))
                for bi, bank in enumerate(ost["banks"]):
                    oc = ocp.tile([128, IBS], F32, tag="oc", name="oc")
                    nc.vector.tensor_copy(oc[:], bank[:])  # releases the bank
                    rsh = rshp.tile([128, IBS], F32, tag="rsh", name="rsh")
                    nc.vector.memset(rsh[:], 1.0)
                    nc.sync.dma_start(rsh[0:32, :], oc[32:64, :])
                    nc.sync.dma_start(rsh[64:96, :], oc[96:128, :])
                    rr = rshp.tile([128, IBS], F32, tag="rr", name="rr")
                    nc.vector.memset(rr[:], 1.0)
                    nc.vector.reciprocal_approx_fast(rr[:], rsh[:])
                    nc.vector.tensor_mul(outT[2 * g + bi][:, ibs], oc[:], rr[:])
                ost["banks"] = None
                ost["bcount"] = [0, 0]

            def emit_phase5(ib):
                # out-projection + residual for the 4 token tiles of this ib;
                # legal once both head-groups' norms for ib are done.
                for t in range(4 * ib, 4 * ib + 4):
                    ts_ = slice(128 * t, 128 * (t + 1))
                    f_ps = slot(t % 2)
                    for p in range(4):
                        nc.tensor.matmul(f_ps[:, :DIM], lhsT=outT[p][:, ts_],
                                         rhs=wp[p][:], start=(p == 0), stop=False)
                    nc.tensor.matmul(f_ps[:, :DIM], lhsT=onerf[:], rhs=bp[:],
                                     start=False, stop=True)
                    f_sb = fp.tile([128, DIM], F32, tag="f", name="f")
                    nc.vector.tensor_add(f_sb[:], f_ps[:, :DIM], xsb[t][:])
                    nc.sync.dma_start(out_d[ts_, :], f_sb[:])

            def drain_O():
                # emit O-chunks for the current owner; on completion, norm +
                # fire any queued projection window, then advance the owner.
                while ost["head"] < len(ost["order"]):
                    g, ib = ost["order"][ost["head"]]
                    rl = ost["ready"].get((g, ib), [])
                    done = ost["done"].get((g, ib), 0)
                    for jt, h, pt, off in rl[done:]:
                        emit_O_chunk(g, ib, jt, h, pt, off)
                    ost["done"][(g, ib)] = len(rl)
                    if len(rl) < 4 * JT:
                        return
                    emit_norm(g, ib)
                    for g2, c in ost["windows"].pop((g, ib), []):
                        emit_tpose_qk_v(g2, c)
                    if g == 1:
                        emit_phase5(ib)
                    ost["head"] += 1

            def flush_exp():
                stt, fill = st_state["tile"], st_state["fill"]
                if stt is None or fill == 0:
                    return
                pt = ptp.tile([128, STW * CHUNK], BF16, tag="pt", name="pt")
                nc.scalar.activation(pt[:, :fill * CHUNK], stt[0][:, :fill * CHUNK],
                                     Act.Exp, bias=zeroc[:])
                for (gib, jt, h, off) in stt[1]:
                    ost["ready"].setdefault(gib, []).append((jt, h, pt, off))
                st_state["tile"] = None
                st_state["fill"] = 0
                st_state["cap"] = STW
                drain_O()

            def emit_S(g, ib, jt):
                for h in range(4):
                    if st_state["tile"] is None:
                        st_state["tile"] = (
                            stp.tile([128, STW * CHUNK], F32, tag="st", name="st"), [])
                    stt, fill = st_state["tile"], st_state["fill"]
                    cs = slice(CHUNK * fill, CHUNK * (fill + 1))
                    nc.tensor.matmul(
                        stt[0][:, cs],
                        lhsT=kT[g][32 * h:32 * (h + 1), 128 * jt:128 * (jt + 1)],
                        rhs=qT[g][32 * h:32 * (h + 1), IBS * ib:IBS * (ib + 1)],
                        start=True, stop=True, tile_position=(32 * h, 0))
                    stt[1].append(((g, ib), jt, h, fill))
                    st_state["fill"] = fill + 1
                    if st_state["fill"] == st_state["cap"]:
                        flush_exp()

            def emit_v_pair(ptile, base, t0):
                # v for tokens t0, t0+1 into ptile[:, base:base+512]
                for ti in range(2):
                    t = t0 + ti
                    vs = slice(base + 256 * ti, base + 256 * (ti + 1))
                    nc.tensor.matmul(ptile[:, vs], lhsT=xnT[0][:, 128 * t:128 * (t + 1)],
                                     rhs=wv[0][:], start=True, stop=False)
                    nc.tensor.matmul(ptile[:, vs], lhsT=xnT[1][:, 128 * t:128 * (t + 1)],
                                     rhs=wv[1][:], start=False, stop=False)
                    nc.tensor.matmul(ptile[:, vs], lhsT=onerb[:], rhs=bv[:],
                                     start=False, stop=True)
                vdst = vsb[:, 512 * t0:512 * (t0 + 2)].rearrange(
                    "p (t hh cc) -> p t hh cc", t=2, cc=64)[:, :, :, 0:32]
                vsrc = ptile[:, base:base + 512].rearrange(
                    "p (t hh cc) -> p t hh cc", t=2, cc=32)
                nc.vector.tensor_copy(vdst, vsrc)

            def emit_tpose_qk_v(g2, c):
                # g2: head-group whose qk to project; c: column chunk 0..3.
                # Projections borrow st-pool tiles (512-col bank slices) so the
                # os banks stay exclusive to O accumulation. For g2==0, two
                # tiles per c (even count keeps the S/exp ping-pong parity);
                # window calls (g2==1) come in pairs for the same reason.
                if g2 == 0:
                    xcs = []
                    for t in range(4 * c, 4 * c + 4):
                        xc = xcp.tile([128, DIM], F32, tag="xc", name="xc")
                        nc.vector.tensor_scalar(
                            xc[:], xsb[t][:], stats[:, 2 * t:2 * t + 1],
                            rstd[:, t:t + 1], op0=Alu.subtract, op1=Alu.mult)
                        xcs.append(xc)
                    pt1 = stp.tile([128, STW * CHUNK], F32, tag="st", name="st")
                    for gg in range(2):
                        for t4 in range(4):
                            nc.tensor.transpose(
                                pt1[:, 512 * gg + 128 * t4:512 * gg + 128 * (t4 + 1)],
                                xcs[t4][:, 128 * gg:128 * (gg + 1)],
                                i128[:])
                        nc.vector.tensor_copy(
                            xnT[gg][:, 512 * c:512 * (c + 1)],
                            pt1[:, 512 * gg:512 * (gg + 1)])
                cs = slice(512 * c, 512 * (c + 1))
                pt2 = stp.tile([128, STW * CHUNK], F32, tag="st", name="st")
                for which, (w, bias, dstT) in enumerate(
                        [(wq, bq, qT), (wk, bk, kT)]):
                    pj = pt2[:, 512 * which:512 * (which + 1)]
                    for ki in range(3):
                        rhs = xnT[ki][:, cs] if ki < 2 else posT[:, cs]
                        nc.tensor.matmul(pj, lhsT=w[ki][g2][:], rhs=rhs,
                                         start=(ki == 0), stop=(ki == 2))
                    nc.vector.tensor_scalar(dstT[g2][:, cs], pj, bias[g2][:],
                                            None, op0=Alu.add)
                if g2 == 0:
                    emit_v_pair(pt2, 1024, 4 * c)
                    emit_v_pair(pt1, 1024, 4 * c + 2)

            # ---------------- emission schedule ----------------
            # diagonal growth: round c emits S-chunks with ib + jt//4 == c,
            # so (0,0) completes early and O-draining starts immediately.
            ost["windows"][(0, 0)] = [(1, 0), (1, 1)]
            ost["windows"][(0, 1)] = [(1, 2), (1, 3)]
            for c in range(4):
                emit_tpose_qk_v(0, c)
                for ib in range(min(c + 1, IB)):
                    jq = c - ib
                    for jt in range(4 * jq, 4 * jq + 4):
                        emit_S(0, ib, jt)
            # g0 remainder: ib + jt//4 >= 4, closing ibs in order
            for ib in range(1, IB):
                for jq in range(4 - ib, 4):
                    for jt in range(4 * jq, 4 * jq + 4):
                        emit_S(0, ib, jt)
            # g1 (windows fired inline at close(0,0) and close(0,1))
            for ib in range(2):
                for jt in range(8):
                    emit_S(1, ib, jt)
            for ib in range(2):
                for jt in range(8, JT):
                    emit_S(1, ib, jt)
            for ib in range(2, IB):
                for jt in range(JT):
                    emit_S(1, ib, jt)
            flush_exp()  # phase5 emitted inline by drain_O at each g1 norm

    nc.compile()
    return nc


def make_in_maps(inputs, n=N, nb=B):
    x = np.ascontiguousarray(np.asarray(inputs["x"], np.float32))
    pos = np.asarray(inputs["pos_embed"], np.float32)
    f32 = lambda a: np.ascontiguousarray(np.asarray(a, np.float32))
    bf16 = lambda a: np.ascontiguousarray(np.asarray(a, np.float32).astype(BF16_NP))

    g = f32(inputs["ln_g"]).reshape(DIM)
    b = f32(inputs["ln_b"]).reshape(DIM)
    rs = SCALE ** 0.5  # sqrt of softmax scale, folded into q AND k
    Wq, Wk, Wv = f32(inputs["Wq"]), f32(inputs["Wk"]), f32(inputs["Wv"])
    wq_eff = Wq.copy()
    wq_eff[:DIM] *= g[:, None]
    wk_eff = Wk.copy()
    wk_eff[:DIM] *= g[:, None]
    bq_eff = (f32(inputs["bq"]) + Wq[:DIM].T @ b) * rs
    bk_eff = (f32(inputs["bk"]) + Wk[:DIM].T @ b) * rs
    wv_eff = Wv * g[:, None]
    bv_eff = f32(inputs["bv"]) + Wv.T @ b

    shared = {
        "wq": bf16(wq_eff * rs), "wk": bf16(wk_eff * rs), "wv": bf16(wv_eff),
        "bq": bq_eff.reshape(DIM, 1), "bk": bk_eff.reshape(DIM, 1),
        "bv": bf16(bv_eff).reshape(1, DIM), "bp": f32(inputs["bp"]).reshape(1, DIM),
        "ones_row_bf": np.ones((1, 128), BF16_NP),
        "ones_row_f": np.ones((1, 128), np.float32),
        "i128": np.eye(128, dtype=np.float32),
        "epsc": np.full((128, 1), EPS, np.float32),
        "zeroc": np.zeros((128, 1), np.float32),
    }
    wp_full = f32(inputs["Wp"])
    for p, (ha, hb) in enumerate(PAIR_HEADS):
        pad = np.zeros((128, DIM), np.float32)
        pad[0:32] = wp_full[32 * ha:32 * ha + 32]
        pad[64:96] = wp_full[32 * hb:32 * hb + 32]
        shared[f"wp{p}"] = pad
    in_maps = []
    for bb in range(nb):
        m = dict(shared)
        m["x"] = np.ascontiguousarray(x[bb, :n])
        m["posT"] = np.ascontiguousarray(pos[bb, :n].T.astype(BF16_NP))
        in_maps.append(m)
    return in_maps


_NC_CACHE = {}


def kernel(**inputs):
    if "nc" not in _NC_CACHE:
        _NC_CACHE["nc"] = build_nc()
    nc = _NC_CACHE["nc"]
    in_maps = make_in_maps(inputs)
    trace = bool(int(os.environ.get("KERNEL_TRACE", "0")))
    if not trace:
        os.environ["BASS_NEVER_TRACE"] = "1"
    res = bass_utils.run_bass_kernel_spmd(nc, in_maps, core_ids=list(range(B)), trace=trace)
    kernel.last_results = res
    kernel.last_exec_time_ns = res.exec_time_ns
    return np.stack([r["out"] for r in res.results]).astype(np.float32)


# revision 5
# speedup vs baseline: 1.1573x; 1.1375x over previous
"""Trainium2 Bass kernel v2 for nn_Attention_44143673868291.

Data-parallel over batch (core b = batch b), no collectives. The softmax
exp on the ACT engine is the wall (~33.5M exps/core at ~4 elem/cy/lane);
everything else is scheduled to hide under it. Differences vs v1:
  - exp tiles are [128,1536] (3 psum banks, x2 ping-pong = 6 banks):
    ~170 ACT instructions instead of 256 -> less per-instruction
    overhead on the bottleneck engine.
  - O/r psum pair (2 banks, memset-0 + start=False accumulation) drains
    UNNORMALIZED to SBUF (bank released after one copy); recip + divide
    deferred off the bank-critical path; r-row alignment via a tiny
    SBUF->SBUF DMA partition shift (no PE shift-matmul, no psum hold).
  - out-projection in float32r (1 cy/row at FD>=256, ~fp32-mult
    precision; fp32 was 4 cy/row, bf16 lost 2e-2-level accuracy).
  - rstd = exp(-0.5*ln(var+eps)) + one DVE Newton step: Ln+Exp share
    one ACT table set (no Sqrt table switch; exp tables load during LN).
  - diagonal S-chunk schedule (ib + jt//4 == c round order): first exp
    fires ~4us in, right after the first q/k projection chunk; O-chunks
    drain incrementally per (g, ib) owner of the 2 os banks; qk(g1)
    projections fire in windows at (0,0)/(0,1) close; per-i-block
    out-projection runs inline after each g1 norm (tail hidden).
  - projections borrow st-pool psum tiles (512-col bank slices), in
    pairs to preserve the S/exp ping-pong buffer parity.
  - gamma/beta and sqrt(softmax scale) folded into weights host-side.
"""

import os
import sys

_REPO = "/opt/trn_rl_repo"
if _REPO not in sys.path:
    sys.path.insert(0, _REPO)

import numpy as np
import ml_dtypes

import concourse.bass as bass
import concourse.mybir as mybir
import concourse.bacc as bacc
import concourse.tile as tile
from concourse import bass_utils

F32 = mybir.dt.float32
F32R = mybir.dt.float32r
BF16 = mybir.dt.bfloat16
BF16_NP = ml_dtypes.bfloat16
Alu = mybir.AluOpType
Act = mybir.ActivationFunctionType

B, N, DIM, POS, H = 8, 2048, 256, 128, 8
QK = DIM + POS  # 384
HD = DIM // H   # 32
SCALE = HD ** -0.5
EPS = 1e-5
IBS = 512
TT = N // 128   # 16 token tiles
JT = N // 128   # 16 j tiles
IB = N // IBS   # 4 i-blocks per head-group
CHUNK = 512     # columns per S-chunk (one head x one i-block x one j-tile)
STW = 3         # S-chunks per exp tile (1536 cols)

# bank pair layout: bankA holds heads (4g+0, 4g+2), bankB holds (4g+3, 4g+1)
#   h0->(A,0) h1->(B,64) h2->(A,64) h3->(B,0)
H_BANK = (0, 1, 0, 1)   # which bank (A=0/B=1) per h%4
H_POS = (0, 64, 64, 0)  # column/tile position per h%4
PAIR_HEADS = [(0, 2), (3, 1), (4, 6), (7, 5)]


def build_nc(n=N, repeat=1, ptbufs=26):
    nc = bacc.Bacc("TRN2", target_bir_lowering=False, debug=False)

    d = lambda name, shape, dt: nc.dram_tensor(name, shape, dt, kind="ExternalInput").ap()
    x_d = d("x", [n, DIM], F32)
    posT_d = d("posT", [POS, n], BF16)
    wq_d = d("wq", [QK, DIM], BF16)
    wk_d = d("wk", [QK, DIM], BF16)
    wv_d = d("wv", [DIM, DIM], BF16)
    wp_d = [d(f"wp{p}", [128, DIM], F32R) for p in range(4)]
    bq_d = d("bq", [DIM, 1], F32)
    bk_d = d("bk", [DIM, 1], F32)
    bv_d = d("bv", [1, DIM], BF16)
    bp_d = d("bp", [1, DIM], F32R)
    onerb_d = d("ones_row_bf", [1, 128], BF16)
    onerf_d = d("ones_row_f", [1, 128], F32R)
    i128_d = d("i128", [128, 128], F32)
    epsc_d = d("epsc", [128, 1], F32)
    zeroc_d = d("zeroc", [128, 1], F32)
    out_d = nc.dram_tensor("out", [n, DIM], F32, kind="ExternalOutput").ap()

    from contextlib import ExitStack

    with tile.TileContext(nc) as tc, ExitStack() as ctx:
        cp = ctx.enter_context(tc.tile_pool(name="const", bufs=1))

        def ctile(shape, dt, src, tag):
            t = cp.tile(shape, dt, tag=tag, name=tag)
            nc.sync.dma_start(t[:], src)
            return t

        # DMA order = need-time: x[0:4] (LN) -> early consts (transposes,
        # qk projections) -> rest of x -> late consts.
        xsb = [cp.tile([128, DIM], F32, tag=f"x{t}", name=f"x{t}") for t in range(TT)]
        for t in range(4):
            nc.sync.dma_start(xsb[t][:], x_d[128 * t:128 * (t + 1), :])
        epsc = ctile([128, 1], F32, epsc_d[:, :], "epsc")
        zeroc = ctile([128, 1], F32, zeroc_d[:, :], "zeroc")
        i128 = ctile([128, 128], F32, i128_d[:, :], "i128")
        wq = [[ctile([128, 128], BF16, wq_d[128 * k:128 * (k + 1), 128 * g:128 * (g + 1)],
                     f"wq{k}{g}") for g in range(2)] for k in range(3)]
        wk = [[ctile([128, 128], BF16, wk_d[128 * k:128 * (k + 1), 128 * g:128 * (g + 1)],
                     f"wk{k}{g}") for g in range(2)] for k in range(3)]
        bq = [ctile([128, 1], F32, bq_d[128 * g:128 * (g + 1), :], f"bq{g}") for g in range(2)]
        bk = [ctile([128, 1], F32, bk_d[128 * g:128 * (g + 1), :], f"bk{g}") for g in range(2)]
        posT = ctile([POS, n], BF16, posT_d[:, :], "posT")
        for t in range(4, TT):
            nc.sync.dma_start(xsb[t][:], x_d[128 * t:128 * (t + 1), :])
        wv = [ctile([128, DIM], BF16, wv_d[128 * k:128 * (k + 1), :], f"wv{k}") for k in range(2)]
        bv = ctile([1, DIM], BF16, bv_d[:, :], "bv")
        zer128 = cp.tile([128, 128], BF16, tag="zer128", name="zer128")
        nc.vector.memset(zer128[:], 0.0)
        onerb = ctile([1, 128], BF16, onerb_d[:, :], "onerb")
        onerf = ctile([1, 128], F32R, onerf_d[:, :], "onerf")
        wp = [ctile([128, DIM], F32R, wp_d[p][:, :], f"wp{p}") for p in range(4)]
        bp = ctile([1, DIM], F32R, bp_d[:, :], "bp")

        # persistent activations
        xnT = [cp.tile([128, n], BF16, tag=f"xnT{g}", name=f"xnT{g}") for g in range(2)]
        qT = [cp.tile([128, n], BF16, tag=f"qT{g}", name=f"qT{g}") for g in range(2)]
        kT = [cp.tile([128, n], BF16, tag=f"kT{g}", name=f"kT{g}") for g in range(2)]
        # all-t augmented V: col = 512*t + 64*h + c, c<32 = v, c>=32 = ones
        vsb = cp.tile([128, TT * 512], BF16, tag="vsb", name="vsb")
        nc.vector.memset(vsb[:], 1.0)
        outT = [cp.tile([128, n], F32R, tag=f"outT{p}", name=f"outT{p}") for p in range(4)]
        stats = cp.tile([128, 2 * TT], F32, tag="stats", name="stats")
        lnv = cp.tile([128, TT], F32, tag="lnv", name="lnv")
        rstd = cp.tile([128, TT], F32, tag="rstd", name="rstd")
        vz = cp.tile([128, TT], F32, tag="vz", name="vz")
        nt1 = cp.tile([128, TT], F32, tag="nt1", name="nt1")
        nt2 = cp.tile([128, TT], F32, tag="nt2", name="nt2")

        bn6p = ctx.enter_context(tc.tile_pool(name="bn6", bufs=3))
        xcp = ctx.enter_context(tc.tile_pool(name="xc", bufs=6))
        ptp = ctx.enter_context(tc.tile_pool(name="pt", bufs=ptbufs))
        ocp = ctx.enter_context(tc.tile_pool(name="oc", bufs=2))
        rshp = ctx.enter_context(tc.tile_pool(name="rsh", bufs=2))
        fp = ctx.enter_context(tc.tile_pool(name="fout", bufs=3))
        # psum: slots 2 banks + st 6 banks = 8
        op = ctx.enter_context(tc.tile_pool(name="slots", bufs=1, space="PSUM"))
        stp = ctx.enter_context(tc.tile_pool(name="st", bufs=2, space="PSUM"))

        slot = lambda i: op.tile([128, 512], F32, tag=f"s{'AB'[i]}", name=f"s{'AB'[i]}")

        for _rep in range(repeat):
            # ---------------- LN stats ----------------
            for t in range(TT):
                if _rep > 0:
                    nc.sync.dma_start(xsb[t][:], x_d[128 * t:128 * (t + 1), :])
                b6 = bn6p.tile([128, 6], F32, tag="b6", name="b6")
                nc.vector.bn_stats(b6[:], xsb[t][:])
                nc.vector.bn_aggr(stats[:, 2 * t:2 * t + 2], b6[:])
            # rstd = exp(-0.5*ln(var+eps)); Ln+Exp share one ACT table set,
            # and this loads the exp tables before the big softmax exps.
            QB = max(TT // 4, 1)
            for qb in range(0, TT, QB):
                qn = min(QB, TT - qb)
                var_v = stats[:, 2 * qb:2 * (qb + qn)].rearrange(
                    "p (t c) -> p t c", c=2)[:, :, 1:2]
                lnv_v = lnv[:, qb:qb + qn].rearrange("p (t c) -> p t c", c=1)
                rstd_v = rstd[:, qb:qb + qn].rearrange("p (t c) -> p t c", c=1)
                nc.scalar.activation(lnv_v, var_v, Act.Ln, bias=epsc[:])
                nc.scalar.activation(rstd_v, lnv_v, Act.Exp, bias=zeroc[:], scale=-0.5)
                vz_v = vz[:, qb:qb + qn].rearrange("p (t c) -> p t c", c=1)
                nc.vector.tensor_scalar(vz_v, var_v, epsc[:], None, op0=Alu.add)
                # per-quarter Newton step squares away the ACT-table error
                # (per-quarter so xc(t0..3) doesn't wait on the full LN):
                # rstd <- rstd * (1.5 - 0.5 * (var+eps) * rstd^2)
                q_ = slice(qb, qb + qn)
                nc.vector.tensor_mul(nt1[:, q_], rstd[:, q_], rstd[:, q_])
                nc.vector.tensor_mul(nt1[:, q_], nt1[:, q_], vz[:, q_])
                nc.vector.tensor_mul(nt1[:, q_], nt1[:, q_], rstd[:, q_])
                nc.vector.tensor_scalar(nt2[:, q_], rstd[:, q_], 1.5, None, op0=Alu.mult)
                nc.vector.tensor_scalar(nt1[:, q_], nt1[:, q_], 0.5, None, op0=Alu.mult)
                nc.vector.tensor_sub(rstd[:, q_], nt2[:, q_], nt1[:, q_])

            # ---------------- chunk/exp stream machinery ----------------
            st_state = {"tile": None, "fill": 0, "cap": 1}  # first tile: 1 chunk
            # O-phase state: one (g, ib) pair owns the two os slot banks at a
            # time; its O-chunks are emitted incrementally as exp tiles flush.
            ost = {
                "order": [(g, ib) for g in range(2) for ib in range(IB)],
                "head": 0,            # index into order: current O owner
                "ready": {},          # (g, ib) -> list of (jt, h, pt, off)
                "done": {},           # (g, ib) -> count of emitted O-chunks
                "banks": None,        # (osA, osB) tiles of current owner
                "bcount": [0, 0],     # chunks emitted per bank for owner
                "windows": {},        # close-of-(g,ib) -> list of (g2, c) qk
                "rsh_init": 0,        # rsh buffers 1.0-initialized so far
            }

            def emit_O_chunk(g, ib, jt, h, pt, off):
                if ost["banks"] is None:
                    ost["banks"] = (slot(0), slot(1))
                    # zero the banks on the PE (start=True + zero weights
                    # writes a full-bank 0 and sets has_written); DVE memsets
                    # here cost 2x658ns each on the busier engine
                    for bk_ in ost["banks"]:
                        nc.tensor.matmul(bk_[:], lhsT=zer128[:],
                                         rhs=vsb[:, 0:512], start=True,
                                         stop=False, skip_group_check=True)
                bi, pos = H_BANK[h], H_POS[h]
                ost["bcount"][bi] += 1
                nc.tensor.matmul(
                    ost["banks"][bi][pos:pos + 64, :],
                    lhsT=vsb[:, 512 * jt + 64 * (4 * g + h):
                             512 * jt + 64 * (4 * g + h) + 64],
                    rhs=pt[:, CHUNK * off:CHUNK * (off + 1)],
                    start=False,
                    stop=ost["bcount"][bi] == 2 * JT,
                    tile_position=(0, pos), skip_group_check=True)

            def emit_norm(g, ib):
                ibs = slice(IBS * ib, IBS * (ib + 1))
                for bi, bank in enumerate(ost["banks"]):
                    oc = ocp.tile([128, IBS], F32, tag="oc", name="oc")
                    nc.vector.tensor_copy(oc[:], bank[:])  # releases the bank
                    rsh = rshp.tile([128, IBS], F32, tag="rsh", name="rsh")
                    # junk rows (32-63, 96-127) are only ever written by this
                    # memset; later allocations of the same buffer still hold
                    # 1.0 there, so first-touch-per-buffer suffices
                    if ost["rsh_init"] < 2:
                        nc.vector.memset(rsh[:], 1.0)
                        ost["rsh_init"] += 1
                    nc.sync.dma_start(rsh[0:32, :], oc[32:64, :])
                    nc.sync.dma_start(rsh[64:96, :], oc[96:128, :])
                    rr = rshp.tile([128, IBS], F32, tag="rr", name="rr")
                    nc.vector.reciprocal_approx_fast(rr[:], rsh[:])
                    nc.vector.tensor_mul(outT[2 * g + bi][:, ibs], oc[:], rr[:])
                ost["banks"] = None
                ost["bcount"] = [0, 0]

            def emit_phase5(ib):
                # out-projection + residual for the 4 token tiles of this ib;
                # legal once both head-groups' norms for ib are done.
                for t in range(4 * ib, 4 * ib + 4):
                    ts_ = slice(128 * t, 128 * (t + 1))
                    f_ps = slot(t % 2)
                    for p in range(4):
                        nc.tensor.matmul(f_ps[:, :DIM], lhsT=outT[p][:, ts_],
                                         rhs=wp[p][:], start=(p == 0), stop=False)
                    nc.tensor.matmul(f_ps[:, :DIM], lhsT=onerf[:], rhs=bp[:],
                                     start=False, stop=True)
                    f_sb = fp.tile([128, DIM], F32, tag="f", name="f")
                    nc.vector.tensor_add(f_sb[:], f_ps[:, :DIM], xsb[t][:])
                    nc.sync.dma_start(out_d[ts_, :], f_sb[:])

            def drain_O():
                # emit O-chunks for the current owner; on completion, norm +
                # fire any queued projection window, then advance the owner.
                while ost["head"] < len(ost["order"]):
                    g, ib = ost["order"][ost["head"]]
                    rl = ost["ready"].get((g, ib), [])
                    done = ost["done"].get((g, ib), 0)
                    for jt, h, pt, off in rl[done:]:
                        emit_O_chunk(g, ib, jt, h, pt, off)
                    ost["done"][(g, ib)] = len(rl)
                    if len(rl) < 4 * JT:
                        return
                    emit_norm(g, ib)
                    for g2, c in ost["windows"].pop((g, ib), []):
                        emit_tpose_qk_v(g2, c)
                    if g == 1:
                        emit_phase5(ib)
                    ost["head"] += 1

            def flush_exp():
                stt, fill = st_state["tile"], st_state["fill"]
                if stt is None or fill == 0:
                    return
                pt = ptp.tile([128, STW * CHUNK], BF16, tag="pt", name="pt")
                nc.scalar.activation(pt[:, :fill * CHUNK], stt[0][:, :fill * CHUNK],
                                     Act.Exp, bias=zeroc[:])
                for (gib, jt, h, off) in stt[1]:
                    ost["ready"].setdefault(gib, []).append((jt, h, pt, off))
                st_state["tile"] = None
                st_state["fill"] = 0
                st_state["cap"] = STW
                drain_O()

            def emit_S(g, ib, jt):
                for h in range(4):
                    if st_state["tile"] is None:
                        st_state["tile"] = (
                            stp.tile([128, STW * CHUNK], F32, tag="st", name="st"), [])
                    stt, fill = st_state["tile"], st_state["fill"]
                    cs = slice(CHUNK * fill, CHUNK * (fill + 1))
                    nc.tensor.matmul(
                        stt[0][:, cs],
                        lhsT=kT[g][32 * h:32 * (h + 1), 128 * jt:128 * (jt + 1)],
                        rhs=qT[g][32 * h:32 * (h + 1), IBS * ib:IBS * (ib + 1)],
                        start=True, stop=True, tile_position=(32 * h, 0))
                    stt[1].append(((g, ib), jt, h, fill))
                    st_state["fill"] = fill + 1
                    if st_state["fill"] == st_state["cap"]:
                        flush_exp()

            def emit_v_pair(ptile, base, t0):
                # v for tokens t0, t0+1 into ptile[:, base:base+512]
                for ti in range(2):
                    t = t0 + ti
                    vs = slice(base + 256 * ti, base + 256 * (ti + 1))
                    nc.tensor.matmul(ptile[:, vs], lhsT=xnT[0][:, 128 * t:128 * (t + 1)],
                                     rhs=wv[0][:], start=True, stop=False)
                    nc.tensor.matmul(ptile[:, vs], lhsT=xnT[1][:, 128 * t:128 * (t + 1)],
                                     rhs=wv[1][:], start=False, stop=False)
                    nc.tensor.matmul(ptile[:, vs], lhsT=onerb[:], rhs=bv[:],
                                     start=False, stop=True)
                vdst = vsb[:, 512 * t0:512 * (t0 + 2)].rearrange(
                    "p (t hh cc) -> p t hh cc", t=2, cc=64)[:, :, :, 0:32]
                vsrc = ptile[:, base:base + 512].rearrange(
                    "p (t hh cc) -> p t hh cc", t=2, cc=32)
                nc.vector.tensor_copy(vdst, vsrc)

            def emit_tpose_qk_v(g2, c):
                # g2: head-group whose qk to project; c: column chunk 0..3.
                # Projections borrow st-pool tiles (512-col bank slices) so the
                # os banks stay exclusive to O accumulation. For g2==0, two
                # tiles per c (even count keeps the S/exp ping-pong parity);
                # window calls (g2==1) come in pairs for the same reason.
                if g2 == 0:
                    xcs = []
                    for t in range(4 * c, 4 * c + 4):
                        xc = xcp.tile([128, DIM], F32, tag="xc", name="xc")
                        nc.vector.tensor_scalar(
                            xc[:], xsb[t][:], stats[:, 2 * t:2 * t + 1],
                            rstd[:, t:t + 1], op0=Alu.subtract, op1=Alu.mult)
                        xcs.append(xc)
                    pt1 = stp.tile([128, STW * CHUNK], F32, tag="st", name="st")
                    for gg in range(2):
                        for t4 in range(4):
                            nc.tensor.transpose(
                                pt1[:, 512 * gg + 128 * t4:512 * gg + 128 * (t4 + 1)],
                                xcs[t4][:, 128 * gg:128 * (gg + 1)],
                                i128[:])
                        nc.vector.tensor_copy(
                            xnT[gg][:, 512 * c:512 * (c + 1)],
                            pt1[:, 512 * gg:512 * (gg + 1)])
                cs = slice(512 * c, 512 * (c + 1))
                pt2 = stp.tile([128, STW * CHUNK], F32, tag="st", name="st")
                for which, (w, bias, dstT) in enumerate(
                        [(wq, bq, qT), (wk, bk, kT)]):
                    pj = pt2[:, 512 * which:512 * (which + 1)]
                    for ki in range(3):
                        rhs = xnT[ki][:, cs] if ki < 2 else posT[:, cs]
                        nc.tensor.matmul(pj, lhsT=w[ki][g2][:], rhs=rhs,
                                         start=(ki == 0), stop=(ki == 2))
                    nc.vector.tensor_scalar(dstT[g2][:, cs], pj, bias[g2][:],
                                            None, op0=Alu.add)
                if g2 == 0:
                    emit_v_pair(pt2, 1024, 4 * c)
                    emit_v_pair(pt1, 1024, 4 * c + 2)

            # ---------------- emission schedule ----------------
            # diagonal growth: round c emits S-chunks with ib + jt//4 == c,
            # so (0,0) completes early and O-draining starts immediately.
            ost["windows"][(0, 0)] = [(1, 0), (1, 1)]
            ost["windows"][(0, 1)] = [(1, 2), (1, 3)]
            for c in range(4):
                emit_tpose_qk_v(0, c)
                for ib in range(min(c + 1, IB)):
                    jq = c - ib
                    for jt in range(4 * jq, 4 * jq + 4):
                        emit_S(0, ib, jt)
            # g0 remainder: ib + jt//4 >= 4, closing ibs in order
            for ib in range(1, IB):
                for jq in range(4 - ib, 4):
                    for jt in range(4 * jq, 4 * jq + 4):
                        emit_S(0, ib, jt)
            # g1 (windows fired inline at close(0,0) and close(0,1))
            for ib in range(2):
                for jt in range(8):
                    emit_S(1, ib, jt)
            for ib in range(2):
                for jt in range(8, JT):
                    emit_S(1, ib, jt)
            for ib in range(2, IB):
                for jt in range(JT):
                    emit_S(1, ib, jt)
            flush_exp()  # phase5 emitted inline by drain_O at each g1 norm

    nc.compile()
    return nc


def make_in_maps(inputs, n=N, nb=B):
    x = np.ascontiguousarray(np.asarray(inputs["x"], np.float32))
    pos = np.asarray(inputs["pos_embed"], np.float32)
    f32 = lambda a: np.ascontiguousarray(np.asarray(a, np.float32))
    bf16 = lambda a: np.ascontiguousarray(np.asarray(a, np.float32).astype(BF16_NP))

    g = f32(inputs["ln_g"]).reshape(DIM)
    b = f32(inputs["ln_b"]).reshape(DIM)
    rs = SCALE ** 0.5  # sqrt of softmax scale, folded into q AND k
    Wq, Wk, Wv = f32(inputs["Wq"]), f32(inputs["Wk"]), f32(inputs["Wv"])
    wq_eff = Wq.copy()
    wq_eff[:DIM] *= g[:, None]
    wk_eff = Wk.copy()
    wk_eff[:DIM] *= g[:, None]
    bq_eff = (f32(inputs["bq"]) + Wq[:DIM].T @ b) * rs
    bk_eff = (f32(inputs["bk"]) + Wk[:DIM].T @ b) * rs
    wv_eff = Wv * g[:, None]
    bv_eff = f32(inputs["bv"]) + Wv.T @ b

    shared = {
        "wq": bf16(wq_eff * rs), "wk": bf16(wk_eff * rs), "wv": bf16(wv_eff),
        "bq": bq_eff.reshape(DIM, 1), "bk": bk_eff.reshape(DIM, 1),
        "bv": bf16(bv_eff).reshape(1, DIM), "bp": f32(inputs["bp"]).reshape(1, DIM),
        "ones_row_bf": np.ones((1, 128), BF16_NP),
        "ones_row_f": np.ones((1, 128), np.float32),
        "i128": np.eye(128, dtype=np.float32),
        "epsc": np.full((128, 1), EPS, np.float32),
        "zeroc": np.zeros((128, 1), np.float32),
    }
    wp_full = f32(inputs["Wp"])
    for p, (ha, hb) in enumerate(PAIR_HEADS):
        pad = np.zeros((128, DIM), np.float32)
        pad[0:32] = wp_full[32 * ha:32 * ha + 32]
        pad[64:96] = wp_full[32 * hb:32 * hb + 32]
        shared[f"wp{p}"] = pad
    in_maps = []
    for bb in range(nb):
        m = dict(shared)
        m["x"] = np.ascontiguousarray(x[bb, :n])
        m["posT"] = np.ascontiguousarray(pos[bb, :n].T.astype(BF16_NP))
        in_maps.append(m)
    return in_maps


_NC_CACHE = {}


def kernel(**inputs):
    if "nc" not in _NC_CACHE:
        _NC_CACHE["nc"] = build_nc()
    nc = _NC_CACHE["nc"]
    in_maps = make_in_maps(inputs)
    trace = bool(int(os.environ.get("KERNEL_TRACE", "0")))
    if not trace:
        os.environ["BASS_NEVER_TRACE"] = "1"
    res = bass_utils.run_bass_kernel_spmd(nc, in_maps, core_ids=list(range(B)), trace=trace)
    kernel.last_results = res
    kernel.last_exec_time_ns = res.exec_time_ns
    return np.stack([r["out"] for r in res.results]).astype(np.float32)
